# revision 1
# baseline (speedup 1.0000x reference)
"""Causal single-head attention on 8 Trainium2 NeuronCores.

Shapes (hardcoded per problem spec):
  input_tensor [512, 256, 384] f32, Wq/Wk/Wv [384, 64] f32 -> out [512, 256, 64] f32

Sharding: data-parallel on the batch dim, 64 batches per core, weights
replicated.

Per-batch pipeline on each core (S=256 split into two 128-row blocks,
E=384 split into three 128-row chunks):
  1. DMA x_b [256,384] into SBUF with an f32->f16 cast (SWDGE).
  2. PE-transpose the six 128x128 blocks -> xT [384(3 chunks),256] (f16 PSUM,
     exact) then copy to SBUF.
  3. [kT;vT] = [Wk|Wv].T @ xT -> [128,256] (kT at partitions 0:64, vT at
     64:128); qT = Wq.T @ xT -> [64,256].  f16 inputs, f32 accumulation.
  4. PE-transpose vT back to natural v [256,64], append two ones columns
     (col 64 gives the softmax denominator inside the AV matmul).
  5. sT[k,q] = kT_block.T @ qT for both k blocks -> [128,2,256] f32 PSUM.
  6. p = exp(0.125 * sT) on ScalarE (no max subtraction: scores ~ N(0,1), the
     softmax is shift-invariant and exp stays in range), then multiply the two
     diagonal blocks by an upper-triangular 0/1 mask (k<=q).
  7. out_unnorm[q,:] = p_block.T @ [v|1|1], accumulated over the causal k
     blocks only; col 64 = sum_k p = softmax denominator l.
  8. out = out_unnorm[:, :64] * (1/l) in f32, DMA to HBM.

All matmul inputs are fp16 (1 cycle/row on the PE = 4x the fp32 rate, fast
weight loads); every contraction accumulates in f32 PSUM, and the final
normalize runs in f32.
"""

import numpy as np

import concourse.bass as bass
import concourse.mybir as mybir
import concourse.tile as tile
from concourse import bacc
from concourse.bass import ds, ts
from concourse.bass_utils import run_bass_kernel_spmd
from concourse.masks import make_identity, make_upper_triangular

EMBED = 384
HEAD_DIM = 64
SEQ = 256
BATCH = 512
NCORES = 8
NB = BATCH // NCORES  # batches per core

F32 = mybir.dt.float32
F16 = mybir.dt.float16
BF16 = mybir.dt.bfloat16

EC = EMBED // 128  # 3 embed chunks
ST = SEQ // 128    # 2 seq blocks


def _build(nb=NB, mm_dt="f16"):
    """Build the per-core Bass program for nb batches (processed in pairs)."""
    MD = {"f16": F16, "bf16": BF16}[mm_dt]
    assert nb % 2 == 0
    GB = 2               # batches per group
    GS = GB * SEQ        # 512: grouped seq columns
    ng = nb // GB

    nc = bacc.Bacc("TRN2", target_bir_lowering=False)
    x = nc.dram_tensor("x", [nb, SEQ, EMBED], F32, kind="ExternalInput")
    wq = nc.dram_tensor("wq", [EMBED, HEAD_DIM], F32, kind="ExternalInput")
    wk = nc.dram_tensor("wk", [EMBED, HEAD_DIM], F32, kind="ExternalInput")
    wv = nc.dram_tensor("wv", [EMBED, HEAD_DIM], F32, kind="ExternalInput")
    out = nc.dram_tensor("out", [nb, SEQ, HEAD_DIM], F32, kind="ExternalOutput")

    xv = x[:, :, :].rearrange("(g b) (t p) e -> g p b t e", b=GB, p=128)
    ov = out[:, :, :].rearrange("(g b) (t p) d -> g p b t d", b=GB, p=128)

    with tile.TileContext(nc) as tc:
        with (
            tc.tile_pool(name="const", bufs=1) as cpool,
            tc.tile_pool(name="sb_x", bufs=4) as sb_x,
            tc.tile_pool(name="sb_xt", bufs=4) as sb_xt,
            tc.tile_pool(name="sb_qk", bufs=4) as sb_qk,
            tc.tile_pool(name="sb_v", bufs=4) as sb_v,
            tc.tile_pool(name="sb_p", bufs=4) as sb_p,
            tc.tile_pool(name="sb_o", bufs=4) as sb_o,
            tc.tile_pool(name="ps_xt", bufs=1, space="PSUM") as ps_xt,
            tc.tile_pool(name="ps_kv", bufs=1, space="PSUM") as ps_kv,
            tc.tile_pool(name="ps_q", bufs=1, space="PSUM") as ps_q,
            tc.tile_pool(name="ps_vn", bufs=1, space="PSUM") as ps_vn,
            tc.tile_pool(name="ps_st", bufs=2, space="PSUM") as ps_st,
            tc.tile_pool(name="ps_av", bufs=1, space="PSUM") as ps_av,
        ):
            ident = cpool.tile([128, 128], MD)
            make_identity(nc, ident)
            # tri[k, q] = 1.0 where k <= q else 0.0
            tri = cpool.tile([128, 128], MD)
            make_upper_triangular(nc, tri, val=1.0, diag=True)
            # [1, 2] free-broadcast view of tri for the merged mask multiply
            tri_b = bass.AP(
                tensor=tri.tensor,
                offset=tri.offset,
                ap=[tri.ap[0], [0, 2], [1, 128]],
            )

            # [Wk|Wv] packed: projection puts kT at partitions 0:64 (base 0,
            # as the scores matmul needs) and vT at 64:128 (only feeds the
            # PE transpose, which works at base 64 with ident[64:,64:]).
            # gpsimd DMA casts f32 -> f16 on the fly.
            wkv_sb = cpool.tile([128, EC, 128], MD)
            nc.gpsimd.dma_start(
                out=wkv_sb[:, :, 0:HEAD_DIM],
                in_=wk[:, :].rearrange("(c p) d -> p c d", p=128),
            )
            nc.gpsimd.dma_start(
                out=wkv_sb[:, :, HEAD_DIM:128],
                in_=wv[:, :].rearrange("(c p) d -> p c d", p=128),
            )
            wq_sb = cpool.tile([128, EC, HEAD_DIM], MD)
            nc.gpsimd.dma_start(
                out=wq_sb[:, :, :],
                in_=wq[:, :].rearrange("(c p) d -> p c d", p=128),
            )

            AW = HEAD_DIM + 1   # 65: v columns + ones column
            for g in range(ng):
                # 1. load a pair of batches with f32 -> f16 cast
                xs = sb_x.tile([128, GB, ST, EMBED], MD, tag="xs")
                nc.gpsimd.dma_start(out=xs[:, :, :, :], in_=xv[g])

                # 2. transpose x -> xT; block (b,t,c) at col c*512+b*256+t*128
                xt_ps = ps_xt.tile([128, EC * GS], MD, tag="xt")
                for b in range(GB):
                    for t in range(ST):
                        for c in range(EC):
                            nc.tensor.transpose(
                                xt_ps[:, ds(c * GS + b * SEQ + t * 128, 128)],
                                xs[:, b, t, ts(c, 128)],
                                ident[:, :],
                            )
                xts = sb_xt.tile([128, EC, GS], MD, tag="xts")
                nc.vector.tensor_copy(
                    xts[:, :, :],
                    xt_ps[:, :].rearrange("p (c s) -> p c s", c=EC),
                )

                # 3. [kT; vT] and qT projections over both batches (N=512)
                kv_ps = ps_kv.tile([128, GS], F32, tag="kv")
                q_ps = ps_q.tile([HEAD_DIM, GS], F32, tag="q")
                for c in range(EC):
                    nc.tensor.matmul(
                        q_ps[:, :], wq_sb[:, c, :], xts[:, c, :],
                        start=(c == 0), stop=(c == EC - 1),
                    )
                for c in range(EC):
                    nc.tensor.matmul(
                        kv_ps[:, :], wkv_sb[:, c, :], xts[:, c, :],
                        start=(c == 0), stop=(c == EC - 1),
                    )
                qt_sb = sb_qk.tile([HEAD_DIM, GB, SEQ], MD, tag="qt_sb")
                nc.vector.tensor_copy(
                    qt_sb[:, :, :],
                    q_ps[:, :].rearrange("p (b s) -> p b s", b=GB),
                )
                kv_sb = sb_qk.tile([128, GB, SEQ], MD, tag="kv_sb")
                nc.vector.tensor_copy(
                    kv_sb[:, :, :],
                    kv_ps[:, :].rearrange("p (b s) -> p b s", b=GB),
                )

                # 4. transpose vT back to natural v; ones column appended
                vn_ps = ps_vn.tile([128, GB * ST * HEAD_DIM], MD, tag="vn")
                for b in range(GB):
                    for t in range(ST):
                        nc.tensor.transpose(
                            vn_ps[:, ds((b * ST + t) * HEAD_DIM, HEAD_DIM)],
                            kv_sb[HEAD_DIM:128, b, ts(t, 128)],
                            ident[HEAD_DIM:128, HEAD_DIM:128],
                        )
                v_sb = sb_v.tile([128, GB, ST, AW], MD, tag="v_sb")
                nc.vector.tensor_copy(
                    v_sb[:, :, :, 0:HEAD_DIM],
                    vn_ps[:, :].rearrange("p (b t d) -> p b t d", b=GB, t=ST),
                )
                nc.vector.memset(v_sb[:, :, :, HEAD_DIM:AW], 1.0)

                out_sb = sb_o.tile([128, GB, ST, HEAD_DIM], F32, tag="out_sb")
                for b in range(GB):
                    # 5. scores sT[k, q]: k0 vs all q (N=256), k1 vs q1 (N=128)
                    st_ps = ps_st.tile([128, SEQ + 128], F32, tag="st")
                    nc.tensor.matmul(
                        st_ps[:, 0:SEQ],
                        kv_sb[0:HEAD_DIM, b, 0:128],
                        qt_sb[:, b, :],
                        start=True, stop=True,
                    )
                    nc.tensor.matmul(
                        st_ps[:, SEQ : SEQ + 128],
                        kv_sb[0:HEAD_DIM, b, 128:256],
                        qt_sb[:, b, 128:256],
                        start=True, stop=True,
                    )

                    # 6. p = exp(sT/8) in one ACT op; merged causal mask on
                    # the two diagonal blocks (cols 0:128 and 256:384)
                    pt_sb = sb_p.tile([128, SEQ + 128], MD, tag="pt")
                    nc.scalar.activation(
                        pt_sb[:, :],
                        st_ps[:, :],
                        mybir.ActivationFunctionType.Exp,
                        scale=0.125,
                    )
                    diag = bass.AP(
                        tensor=pt_sb.tensor,
                        offset=pt_sb.offset,
                        ap=[pt_sb.ap[0], [SEQ, 2], [1, 128]],
                    )
                    nc.vector.tensor_mul(diag, diag, tri_b)

                    # 7. out_unnorm = p.T @ [v|1]  (col 64 = denominator)
                    av_ps = ps_av.tile([128, 2 * AW], F32, tag="av")
                    nc.tensor.matmul(
                        av_ps[:, 0:AW],
                        pt_sb[:, 0:128], v_sb[:, b, 0, :],
                        start=True, stop=True,
                    )
                    nc.tensor.matmul(
                        av_ps[:, AW : 2 * AW],
                        pt_sb[:, 128:256], v_sb[:, b, 0, :],
                        start=True, stop=False,
                    )
                    nc.tensor.matmul(
                        av_ps[:, AW : 2 * AW],
                        pt_sb[:, 256:384], v_sb[:, b, 1, :],
                        start=False, stop=True,
                    )

                    # 8. normalize rows (f32): one reciprocal + one broadcast
                    # multiply per batch on DVE
                    linv = sb_o.tile([128, ST], F32, tag="linv")
                    avv = av_ps[:, :].rearrange("p (t w) -> p t w", t=ST)
                    nc.vector.reciprocal(
                        linv[:, :], avv[:, :, HEAD_DIM : HEAD_DIM + 1]
                    )
                    linv_b = bass.AP(
                        tensor=linv.tensor,
                        offset=linv.offset,
                        ap=[linv.ap[0], [1, ST], [0, HEAD_DIM]],
                    )
                    nc.vector.tensor_mul(
                        out_sb[:, b, :, :], avv[:, :, 0:HEAD_DIM], linv_b
                    )
                nc.sync.dma_start(out=ov[g], in_=out_sb[:, :, :, :])

    nc.compile()
    return nc


_NC_CACHE = {}


def _get_nc(nb=NB, mm_dt="f16"):
    key = (nb, mm_dt)
    if key not in _NC_CACHE:
        _NC_CACHE[key] = _build(nb, mm_dt)
    return _NC_CACHE[key]


def kernel(input_tensor, Wq, Wk, Wv, **run_kwargs):
    x = np.ascontiguousarray(np.asarray(input_tensor, dtype=np.float32))
    wq = np.ascontiguousarray(np.asarray(Wq, dtype=np.float32))
    wk = np.ascontiguousarray(np.asarray(Wk, dtype=np.float32))
    wv = np.ascontiguousarray(np.asarray(Wv, dtype=np.float32))

    nb = x.shape[0] // NCORES
    nc = _get_nc(nb=nb)
    in_maps = [
        {"x": x[i * nb : (i + 1) * nb], "wq": wq, "wk": wk, "wv": wv}
        for i in range(NCORES)
    ]
    res = run_bass_kernel_spmd(nc, in_maps, core_ids=list(range(NCORES)), **run_kwargs)
    outs = np.concatenate([res.results[i]["out"] for i in range(NCORES)], axis=0)
    if run_kwargs.get("trace"):
        kernel.last_results = res
    return outs

